# revision 2
# baseline (speedup 1.0000x reference)
"""NCNPredictor v6: fp8 adjacency + DMA gather-accumulate + ACT relu mask.

Per 128-edge tile (columns sharded 8-way, 3750 cols/core):
  s   = gather(acat_fp8, i) ; s[:, v] += gather(av_v_fp8, j)   (CCE add, s in {0,1,2})
  t   = Relu(s - 1)  on ScalarE  -> bf16 mask {0,1}, exact
  acc = sum(t * ycat) on DVE, single STT with accum_out
The cn_01*cn_1 / cn_012*cn_01 correction terms (p^4 events, ~0 expected)
are computed exactly on the host and subtracted in combine.
"""

import sys
from contextlib import ExitStack

import numpy as np

sys.path.insert(0, "/opt/trn_rl_repo")

import concourse.bass as bass
import concourse.tile as tile
from concourse import bacc, mybir
from concourse.bass_utils import run_bass_kernel_spmd

N = 10000
D = 128
E = 8192
NCORES = 8
NCOL = N // NCORES          # 1250 columns per view per core
W3 = 3 * NCOL               # 3750
E_OWN = E // NCORES
P = 128
ET = E // P                 # 64 tiles
ET_OWN = E_OWN // P         # 8 tiles
F32 = mybir.dt.float32
BF16 = mybir.dt.bfloat16
FP8 = mybir.dt.float8e4
I32 = mybir.dt.int32
MUL = mybir.AluOpType.mult
ADD = mybir.AluOpType.add

_CACHE = {}


def _build_nc():
    nc = bacc.Bacc(num_swdge_queues=4)

    acat = nc.declare_dram_parameter("acat", [N, W3], FP8, False)
    av = [nc.declare_dram_parameter(f"av{v}", [N, NCOL], FP8, False) for v in range(3)]
    xw = nc.declare_dram_parameter("xw", [N, D], F32, False)
    xr = nc.declare_dram_parameter("xr", [N, D], F32, False)
    ycat = nc.declare_dram_parameter("ycat", [P, W3], BF16, False)
    ii = nc.declare_dram_parameter("ii", [P, ET], I32, False)
    jj = nc.declare_dram_parameter("jj", [P, ET], I32, False)
    iown = nc.declare_dram_parameter("iown", [P, ET_OWN], I32, False)
    jown = nc.declare_dram_parameter("jown", [P, ET_OWN], I32, False)

    out_cn = nc.declare_dram_parameter("out_cn", [P, ET], F32, True)
    out_xij = nc.declare_dram_parameter("out_xij", [P, ET_OWN], F32, True)

    with tile.TileContext(nc) as tc, ExitStack() as ctx:
        const = ctx.enter_context(tc.tile_pool(name="const", bufs=1))
        yc = const.tile([P, W3], BF16)
        nc.sync.dma_start(yc[:], ycat[:])
        ii_t = const.tile([P, ET], I32)
        nc.sync.dma_start(ii_t[:], ii[:])
        jj_t = const.tile([P, ET], I32)
        nc.sync.dma_start(jj_t[:], jj[:])
        io_t = const.tile([P, ET_OWN], I32)
        nc.sync.dma_start(io_t[:], iown[:])
        jo_t = const.tile([P, ET_OWN], I32)
        nc.sync.dma_start(jo_t[:], jown[:])
        neg1 = const.tile([P, 1], F32)
        nc.vector.memset(neg1[:], -1.0)
        acc_cn = const.tile([P, ET], F32)
        acc_xij = const.tile([P, ET_OWN], F32)

        gat = ctx.enter_context(tc.tile_pool(name="gat", bufs=4))
        msk = ctx.enter_context(tc.tile_pool(name="msk", bufs=3))

        for et in range(ET):
            s = gat.tile([P, W3], FP8, name="s")
            nc.gpsimd.indirect_dma_start(
                out=s[:], out_offset=None, in_=acat[:],
                in_offset=bass.IndirectOffsetOnAxis(ap=ii_t[:, et : et + 1], axis=0),
            )
            for v in range(3):
                nc.gpsimd.indirect_dma_start(
                    out=s[:, v * NCOL : (v + 1) * NCOL], out_offset=None,
                    in_=av[v][:],
                    in_offset=bass.IndirectOffsetOnAxis(
                        ap=jj_t[:, et : et + 1], axis=0
                    ),
                    compute_op=ADD,
                )

            t = msk.tile([P, W3], BF16, name="t")
            nc.scalar.activation(
                out=t[:], in_=s[:], func=mybir.ActivationFunctionType.Relu,
                bias=neg1[:], scale=1.0,
            )
            u = msk.tile([P, W3], BF16, name="u")
            nc.vector.scalar_tensor_tensor(
                out=u[:], in0=t[:], scalar=1.0, in1=yc[:],
                op0=MUL, op1=MUL, accum_out=acc_cn[:, et : et + 1],
            )
        nc.sync.dma_start(out_cn[:], acc_cn[:])

        for et in range(ET_OWN):
            xi_t = gat.tile([P, D], F32, name="xi_t")
            nc.gpsimd.indirect_dma_start(
                out=xi_t[:], out_offset=None, in_=xw[:],
                in_offset=bass.IndirectOffsetOnAxis(ap=io_t[:, et : et + 1], axis=0),
            )
            xj_t = gat.tile([P, D], F32, name="xj_t")
            nc.gpsimd.indirect_dma_start(
                out=xj_t[:], out_offset=None, in_=xr[:],
                in_offset=bass.IndirectOffsetOnAxis(ap=jo_t[:, et : et + 1], axis=0),
            )
            oxe = msk.tile([P, D], F32, name="oxe")
            nc.vector.scalar_tensor_tensor(
                out=oxe[:], in0=xi_t[:], scalar=1.0, in1=xj_t[:],
                op0=MUL, op1=MUL, accum_out=acc_xij[:, et : et + 1],
            )
        nc.sync.dma_start(out_xij[:], acc_xij[:])

    return nc


def get_nc():
    if "nc" not in _CACHE:
        nc = _build_nc()
        nc.compile()
        _CACHE["nc"] = nc
    return _CACHE["nc"]


def _to_fp8_binary(a):
    """{0,1} float matrix -> fp8e4m3 bytes without a slow astype."""
    import ml_dtypes

    u8 = (np.asarray(a) != 0).astype(np.uint8) * np.uint8(0x38)  # 0x38 == 1.0
    return u8.view(ml_dtypes.float8_e4m3)


def make_in_maps(x, adj_0_1, adj_1, adj_0_1_2, tar_ei, Wxs, bxs):
    import ml_dtypes

    bf = ml_dtypes.bfloat16
    x32 = np.ascontiguousarray(x, dtype=np.float32)
    wxs = np.asarray(Wxs, dtype=np.float32)
    w0 = wxs[0:D, 0]
    wy = np.concatenate(
        [wxs[D : 2 * D], wxs[2 * D : 3 * D], wxs[3 * D : 4 * D]], axis=1
    )
    y = x32 @ wy                    # [N, 3] per-view node weights
    yb = y.astype(bf)
    xwf = np.ascontiguousarray(x32 * w0[None, :])

    views8 = [_to_fp8_binary(adj_0_1), _to_fp8_binary(adj_1), _to_fp8_binary(adj_0_1_2)]

    ii_all = tar_ei[0].astype(np.int32).reshape(ET, P).T.copy()   # [P, ET]
    jj_all = tar_ei[1].astype(np.int32).reshape(ET, P).T.copy()

    in_maps = []
    for c in range(NCORES):
        c0 = c * NCOL
        acat = np.empty((N, W3), dtype=views8[0].dtype)
        for v in range(3):
            acat[:, v * NCOL : (v + 1) * NCOL] = views8[v][:, c0 : c0 + NCOL]
        ycat = np.empty((P, W3), dtype=bf)
        for v in range(3):
            ycat[:, v * NCOL : (v + 1) * NCOL] = yb[c0 : c0 + NCOL, v][None, :]
        io = tar_ei[0][c * E_OWN : (c + 1) * E_OWN].astype(np.int32)
        jo = tar_ei[1][c * E_OWN : (c + 1) * E_OWN].astype(np.int32)
        in_maps.append({
            "acat": acat,
            "av0": np.ascontiguousarray(views8[0][:, c0 : c0 + NCOL]),
            "av1": np.ascontiguousarray(views8[1][:, c0 : c0 + NCOL]),
            "av2": np.ascontiguousarray(views8[2][:, c0 : c0 + NCOL]),
            "xw": xwf,
            "xr": x32,
            "ycat": ycat,
            "ii": ii_all,
            "jj": jj_all,
            "iown": io.reshape(ET_OWN, P).T.copy(),
            "jown": jo.reshape(ET_OWN, P).T.copy(),
        })
    return in_maps


def host_corrections(adj_0_1, adj_1, adj_0_1_2, tar_ei, Wxs, x):
    """Exact correction: sum_n cn01*cn1*y1[n] + cn012*cn01*y3[n] per edge.

    cn01[e,n]*cn1[e,n] = B1[i,n]*B1[j,n] with B1 = A01 * A1 elementwise.
    B1/B2 have ~p^2*N^2 ~ 400 nonzero entries, so compute sparsely.
    """
    x32 = np.asarray(x, dtype=np.float32)
    wxs = np.asarray(Wxs, dtype=np.float32)
    y1 = x32 @ wxs[D : 2 * D, 0]
    y3 = x32 @ wxs[3 * D : 4 * D, 0]
    i_idx = np.asarray(tar_ei[0])
    j_idx = np.asarray(tar_ei[1])
    corr = np.zeros(E, dtype=np.float64)
    for a, b, yv in ((adj_0_1, adj_1, y1), (adj_0_1_2, adj_0_1, y3)):
        prod = (np.asarray(a) != 0) & (np.asarray(b) != 0)
        rows = np.flatnonzero(prod.any(axis=1))
        if rows.size == 0:
            continue
        rowset = set(rows.tolist())
        cand = [e for e in range(E) if int(i_idx[e]) in rowset and int(j_idx[e]) in rowset]
        for e in cand:
            m = prod[int(i_idx[e])] & prod[int(j_idx[e])]
            if m.any():
                corr[e] += float(yv[m].sum())
    return corr


def combine_results(results, b, corr):
    out = np.zeros((E, 1), dtype=np.float64)
    for c in range(NCORES):
        out[:, 0] += results[c]["out_cn"].astype(np.float64).T.reshape(E)
        sl = slice(c * E_OWN, (c + 1) * E_OWN)
        out[sl, 0] += results[c]["out_xij"].astype(np.float64).T.reshape(E_OWN)
    out[:, 0] -= corr
    return (out + b).astype(np.float32)


def kernel(x, adj_0_1, adj_1, adj_0_1_2, tar_ei, Wxs, bxs):
    nc = get_nc()
    in_maps = make_in_maps(x, adj_0_1, adj_1, adj_0_1_2, tar_ei, Wxs, bxs)
    corr = host_corrections(adj_0_1, adj_1, adj_0_1_2, tar_ei, Wxs, x)
    res = run_bass_kernel_spmd(nc, in_maps, list(range(NCORES)))
    b = float(np.asarray(bxs, dtype=np.float32).reshape(-1)[0])
    return combine_results(res.results, b, corr)
